# revision 34
# baseline (speedup 1.0000x reference)
"""MultiHeadAttention Trainium2 kernel (8 NeuronCores).

Sharding: batch (2) x head-groups (4) -> 8 cores. Core c handles batch c//4
and heads 4*(c%4) .. 4*(c%4)+4 (4 heads of 16, d_k=64 -> 256 of 1024 dims).

Per-core device program (all big matmuls in float32r: full PE rate,
~1.5e-4 rel):
  1. qT/kT = Wqk_slice @ x_b.T, laid out [128, 2, S]: head h lives at
     partitions 64*(h%2) sub h//2, so K=64 score matmuls for head pairs run
     concurrently in separate PE row groups. v natural [S, dk] with an
     appended ones column per head (v_aug) so attn@v also yields the
     softmax normalization sums for free.
  2. scores transposed: sT[j, i] = kT.T @ qT per (head, 128-key tile,
     512-query chunk), two key tiles share a 2-bank PSUM tile; one
     exp(s - 6) ACTIVATE covers [128, 1024] straight from PSUM (constant
     shift instead of row max: scores are O(+-7) here, host-verified).
     Causal masking multiplies diagonal blocks by 2 static 128x1024 keep
     patterns (general mask path multiplies every block).
  3. outT_aug[dk+1, i] = v_aug.T @ p accumulated over key tiles in PSUM;
     row 64 = sums. Normalize: reciprocal_approx_fast (DVE) on the sums
     row, broadcast across partitions with a K=1 f32 matmul, multiply.
  4. Per query chunk: each core computes its full W_o row-slice
     contribution partialT[e, i] with K=256 (its local dims). Host sums
     the 4 partials per batch (the unshard step) -- no device collective.
Host assembles: out[b].T = sum_g partial[4b+g]; out[b] = that transposed.
"""

import sys

sys.path.insert(0, "/opt/trn_rl_repo")

import numpy as np

import concourse.bacc as bacc
import concourse.mybir as mybir
import concourse.tile as tile
from concourse import bass_utils
from concourse.bass import ds, ts

F32 = mybir.dt.float32
F32R = mybir.dt.float32r
EXP = mybir.ActivationFunctionType.Exp

B, S, D = 2, 2048, 1024
H, DK = 16, 64
P = 128
KS = D // P          # 8 contraction subtiles for d=1024
HL = 4               # heads per core
DL = HL * DK         # 256 local d-dims per core
NCI = S // 512       # 4 query chunks
NJT = S // P         # 16 key tiles
N_CORES = 8

_CACHE = {}


def _build(causal: bool):
    nc = bacc.Bacc("TRN2", target_bir_lowering=False, debug=False,
                   num_devices=N_CORES)

    xT_d = nc.dram_tensor("xT", [P, KS, S], F32R, kind="ExternalInput")
    wqk_d = nc.dram_tensor("wqk", [P, KS, 2 * DL], F32R, kind="ExternalInput")
    wv_d = nc.dram_tensor("wv", [P, KS, DL], F32R, kind="ExternalInput")
    wo_d = nc.dram_tensor("wo", [P, 2, D], F32R, kind="ExternalInput")
    if causal:
        # 2 patterns of [128, 2, 512]: diag key-tile pairs vs query chunk
        keep_d = nc.dram_tensor("keep", [P, 2, 2, 512], F32,
                                kind="ExternalInput")
    else:
        keep_d = nc.dram_tensor("keep", [P, NJT, S], F32,
                                kind="ExternalInput")
    out_d = nc.dram_tensor("partialT", [D, S], F32, kind="ExternalOutput")

    with tile.TileContext(nc) as tc:
        with tc.tile_pool(name="persist", bufs=1) as pp:
            qT = pp.tile([P, 2, S], F32R)
            kT = pp.tile([P, 2, S], F32R)
            v_aug = pp.tile([P, NJT, 65 * HL], F32R)
            wo_sb = pp.tile([P, 2, D], F32R)
            attn_outT = pp.tile([P, 2, S], F32R)
            bias_sb = pp.tile([P, 1], F32)
            nc.vector.memset(bias_sb[:], -6.0)
            ones_stage = pp.tile([P, DK], F32)
            nc.vector.memset(ones_stage[:], 1.0)
            for h in range(HL):
                nc.gpsimd.dma_start(v_aug[:, :, 65 * h + DK],
                                    ones_stage[:, 0:NJT])
            if causal:
                keep_sb = pp.tile([P, 2, 2, 512], F32)
                nc.sync.dma_start(keep_sb[:], keep_d[:])

            # ---- Inputs + V projection (k-outer, overlaps input DMA) ----
            ip = tc.alloc_tile_pool(name="inp", bufs=1)
            xT_sb = ip.tile([P, KS, S], F32R)
            wqk_sb = ip.tile([P, KS, 2 * DL], F32R)
            with tc.tile_pool(name="ipv", bufs=1) as ipv, \
                 tc.tile_pool(name="bp", bufs=8, space="PSUM") as bp:
                wv_sb = ipv.tile([P, KS, DL], F32R)
                for k in range(KS):
                    nc.sync.dma_start(wv_sb[:, k, :], wv_d[:, k, :])
                    nc.sync.dma_start(xT_sb[:, k, :], xT_d[:, k, :])
                for k in range(KS):
                    nc.sync.dma_start(wqk_sb[:, k, :], wqk_d[:, k, :])
                nc.sync.dma_start(wo_sb[:], wo_d[:])

                for half in range(2):
                    sts = range(8 * half, 8 * half + 8)
                    pssv = {st: bp.tile([P, DL], F32, tag="b",
                                        name=f"v_{st}")
                            for st in sts}
                    for k in range(KS):
                        for st in sts:
                            nc.tensor.matmul(
                                pssv[st][:],
                                xT_sb[:, k, ts(st, P)],
                                wv_sb[:, k, :],
                                start=(k == 0), stop=(k == KS - 1))
                    for st in sts:
                        for h in range(HL):
                            nc.scalar.copy(v_aug[:, st, ds(65 * h, DK)],
                                           pssv[st][:, ts(h, DK)])

            # ---- Attention + Wo; qk(ci+1) interleaved into attention(ci) ----
            with tc.tile_pool(name="qkp", bufs=2, space="PSUM") as qkp, \
                 tc.tile_pool(name="scp", bufs=2, space="PSUM") as scp, \
                 tc.tile_pool(name="avp", bufs=2, space="PSUM") as avp, \
                 tc.tile_pool(name="ptp", bufs=5 if causal else 4) as ptp, \
                 tc.tile_pool(name="keepp", bufs=4) as keepp, \
                 tc.tile_pool(name="osp", bufs=2) as osp, \
                 tc.tile_pool(name="smp", bufs=3) as smp:

                def emit_qk_group(sc, mc, on_act):
                    ps = qkp.tile([P, 512], F32, tag="q",
                                  name=f"qk_{sc}_{mc}")
                    for k in range(KS):
                        nc.tensor.matmul(
                            ps[:],
                            wqk_sb[:, k, ts(mc, P)],
                            xT_sb[:, k, ts(sc, 512)],
                            start=(k == 0), stop=(k == KS - 1))
                    # mc 0,1 -> q sub 0,1 ; mc 2,3 -> k sub 0,1
                    dst = qT if mc < 2 else kT
                    eng = nc.scalar if on_act else nc.vector
                    if on_act:
                        nc.scalar.copy(dst[:, mc % 2, ts(sc, 512)], ps[:])
                    else:
                        nc.vector.tensor_copy(dst[:, mc % 2, ts(sc, 512)],
                                              ps[:])

                def attention_gen(ci, keep_ci):
                    """Yields after each j2-pair so qk work can interleave."""
                    njt2 = 2 * ci + 2 if causal else NJT // 2
                    for hp in range(2):     # head pairs (2*hp, 2*hp+1)
                        av_pair = [avp.tile([65, 512], F32, tag="av",
                                            name=f"av_{ci}_{hp}_{i}")
                                   for i in range(2)]

                        def emit_av(hh, j2, pt):
                            h = 2 * hp + hh
                            for u in range(2):
                                nc.tensor.matmul(
                                    av_pair[hh][:],
                                    v_aug[:, 2 * j2 + u, ds(65 * h, 65)],
                                    pt[:, u, :],
                                    start=(j2 == 0 and u == 0),
                                    stop=(j2 == njt2 - 1 and u == 1))

                        pend = []
                        for j2 in range(njt2):  # key-tile pairs
                            for hh in range(2):  # head in pair: base 64*hh
                                base = 64 * hh
                                sp = scp.tile([P, 2, 512], F32, tag="sc")
                                for u in range(2):
                                    nc.tensor.matmul(
                                        sp[:, u, :],
                                        kT[ds(base, DK), hp,
                                           ts(2 * j2 + u, P)],
                                        qT[ds(base, DK), hp, ts(ci, 512)],
                                        start=True, stop=True)
                                pt = ptp.tile([P, 2, 512], F32R, tag="p")
                                nc.scalar.activation(pt[:], sp[:], EXP,
                                                     bias=bias_sb[:])
                                if causal:
                                    if j2 >= 2 * ci:
                                        nc.vector.tensor_mul(
                                            pt[:], pt[:],
                                            keep_sb[:, j2 - 2 * ci, :, :])
                                else:
                                    if hh == 0:
                                        keep_blk = keepp.tile(
                                            [P, 2, 512], F32, tag="kb")
                                        nc.sync.dma_start(
                                            keep_blk[:],
                                            keep_d[:, ds(2 * j2, 2),
                                                   ds(ci * 512, 512)])
                                    nc.vector.tensor_mul(pt[:], pt[:],
                                                         keep_blk[:])
                                pend.append((hh, j2, pt))
                            while len(pend) > 4:
                                emit_av(*pend.pop(0))
                            yield
                        for item in pend:
                            emit_av(*item)
                        for hh in range(2):
                            h = 2 * hp + hh
                            av = av_pair[hh]
                            sums = smp.tile([1, 512], F32, tag="sums")
                            nc.vector.tensor_scalar_add(sums[:],
                                                        av[DK:DK + 1, :],
                                                        1e-37)
                            rec = smp.tile([1, 512], F32, tag="rec")
                            nc.vector.reciprocal_approx_fast(rec[:], sums[:])
                            bc_sb = smp.tile([DK, 512], F32, tag="bcs")
                            nc.gpsimd.partition_broadcast(bc_sb[:], rec[:])
                            nc.vector.tensor_mul(
                                attn_outT[ds(64 * (h % 2), DK), h // 2,
                                          ts(ci, 512)],
                                av[0:DK, :], bc_sb[:])
                        yield

                def emit_wo_block(ci, ec):
                    wps = qkp.tile([P, 512], F32, tag="q",
                                   name=f"wo_{ci}_{ec}")
                    for k in range(2):
                        nc.tensor.matmul(
                            wps[:],
                            wo_sb[:, k, ts(ec, P)],
                            attn_outT[:, k, ts(ci, 512)],
                            start=(k == 0), stop=(k == 1))
                    out_sb = osp.tile([P, 512], F32, tag="osb")
                    nc.vector.tensor_copy(out_sb[:], wps[:])
                    nc.sync.dma_start(out_d[ts(ec, P), ts(ci, 512)],
                                      out_sb[:])

                def emit_wo(ci):
                    for ec in range(KS):
                        emit_wo_block(ci, ec)

                if causal:
                    # qk(0) upfront (copies on ACT: attention not running yet)
                    for mc in range(4):
                        emit_qk_group(0, mc, on_act=True)
                    for ci in range(NCI):
                        gen = attention_gen(ci, None)
                        # interleave qk(ci+1) + wo(ci-1) into attention(ci)
                        fill = []
                        if ci > 0:
                            fill += [("wo", ci - 1, ec) for ec in range(KS)]
                        if ci + 1 < NCI:
                            fill += [("qk", ci + 1, mc) for mc in range(4)]
                        fill = fill[1::2] + fill[0::2]  # spread types
                        for _ in gen:
                            if fill:
                                kind, a, b = fill.pop(0)
                                if kind == "qk":
                                    emit_qk_group(a, b, on_act=False)
                                else:
                                    emit_wo_block(a, b)
                        for kind, a, b in fill:
                            if kind == "qk":
                                emit_qk_group(a, b, on_act=False)
                            else:
                                emit_wo_block(a, b)
                    emit_wo(NCI - 1)
                else:
                    for sc in range(4):
                        for mc in range(4):
                            emit_qk_group(sc, mc, on_act=True)
                    for ci in range(NCI):
                        for _ in attention_gen(ci, None):
                            pass
                        emit_wo(ci)
            ip.release()

    nc.compile()
    return nc


def _get(causal: bool):
    if causal not in _CACHE:
        _CACHE[causal] = _build(causal)
    return _CACHE[causal]


def _tile_p(a2d):
    """[R, C] -> [128, R//128, C] with row r at (partition r%128, sub r//128)."""
    r, c = a2d.shape
    return np.ascontiguousarray(
        a2d.reshape(r // P, P, c).transpose(1, 0, 2))


def _causal_patterns():
    """keep[jj, t2, u, ii] for diagonal key-tile-pair t2 (pattern for
    j-tile 2*t2+u within the diag group): keep = ii >= 128*(2*t2+u) + jj."""
    jj = np.arange(P)[:, None, None, None]
    t2 = np.arange(2)[None, :, None, None]
    u = np.arange(2)[None, None, :, None]
    ii = np.arange(512)[None, None, None, :]
    return (ii >= P * (2 * t2 + u) + jj).astype(np.float32)


def _make_in_maps(x, mask, W_q, W_k, W_v, W_o, causal):
    x = np.asarray(x, dtype=np.float32)
    scale = 1.0 / np.sqrt(np.float32(DK))
    if causal:
        keep_host = np.ascontiguousarray(_causal_patterns())
    else:
        keepT = (~np.asarray(mask[0, 0])).astype(np.float32).T
        keep_host = _tile_p(np.ascontiguousarray(keepT))
    in_maps = []
    for c in range(N_CORES):
        b, g = c // 4, c % 4
        sl = slice(g * DL, (g + 1) * DL)
        xT = np.ascontiguousarray(x[b].T)
        # head h -> partitions 64*(h%2), sub h//2: row order within a
        # 256-row slice must be [h0, h1] sub 0 | [h2, h3] sub 1 -> natural.
        wqk = np.concatenate([np.asarray(W_q)[sl] * scale,
                              np.asarray(W_k)[sl]], axis=0).T
        in_maps.append({
            "xT": _tile_p(xT),
            "wqk": _tile_p(np.ascontiguousarray(wqk.astype(np.float32))),
            "wv": _tile_p(np.ascontiguousarray(
                np.asarray(W_v, dtype=np.float32)[sl].T)),
            "wo": _tile_p(np.ascontiguousarray(
                np.asarray(W_o, dtype=np.float32)[:, sl].T)),
            "keep": keep_host,
        })
    return in_maps


def run(x, mask, W_q, W_k, W_v, W_o, trace=False, trace_cores=None):
    mask2d = np.asarray(mask)[0, 0]
    causal = bool(np.array_equal(
        mask2d, ~np.tril(np.ones((S, S), dtype=bool))))
    nc = _get(causal)
    in_maps = _make_in_maps(x, mask, W_q, W_k, W_v, W_o, causal)
    kwargs = {}
    if trace:
        kwargs = dict(trace=True, trace_cores=trace_cores or [0])
    res = bass_utils.run_bass_kernel_spmd(
        nc, in_maps, core_ids=list(range(N_CORES)), **kwargs)
    outs = []
    for b in range(B):
        outT_b = res.results[4 * b]["partialT"].astype(np.float32).copy()
        for g in range(1, 4):
            outT_b += res.results[4 * b + g]["partialT"]
        outs.append(outT_b.T)
    return np.stack(outs).astype(np.float32), res


def kernel(x, mask, W_q, W_k, W_v, W_o):
    out, _ = run(x, mask, W_q, W_k, W_v, W_o, trace=False)
    return out


# revision 35
# speedup vs baseline: 1.0303x; 1.0303x over previous
"""MultiHeadAttention Trainium2 kernel (8 NeuronCores).

Sharding: batch (2) x head-groups (4) -> 8 cores. Core c handles batch c//4
and heads 4*(c%4) .. 4*(c%4)+4 (4 heads of 16, d_k=64 -> 256 of 1024 dims).

Per-core device program (all big matmuls in float32r: full PE rate,
~1.5e-4 rel):
  1. qT/kT = Wqk_slice @ x_b.T, laid out [128, 2, S]: head h lives at
     partitions 64*(h%2) sub h//2, so K=64 score matmuls for head pairs run
     concurrently in separate PE row groups. v natural [S, dk] with an
     appended ones column per head (v_aug) so attn@v also yields the
     softmax normalization sums for free.
  2. scores transposed: sT[j, i] = kT.T @ qT per (head, 128-key tile,
     512-query chunk), two key tiles share a 2-bank PSUM tile; one
     exp(s - 6) ACTIVATE covers [128, 1024] straight from PSUM (constant
     shift instead of row max: scores are O(+-7) here, host-verified).
     Causal masking multiplies diagonal blocks by 2 static 128x1024 keep
     patterns (general mask path multiplies every block).
  3. outT_aug[dk+1, i] = v_aug.T @ p accumulated over key tiles in PSUM;
     row 64 = sums. Normalize: reciprocal_approx_fast (DVE) on the sums
     row, broadcast across partitions with a K=1 f32 matmul, multiply.
  4. Per query chunk: each core computes its full W_o row-slice
     contribution partialT[e, i] with K=256 (its local dims). Host sums
     the 4 partials per batch (the unshard step) -- no device collective.
Host assembles: out[b].T = sum_g partial[4b+g]; out[b] = that transposed.
"""

import sys

sys.path.insert(0, "/opt/trn_rl_repo")

import numpy as np

import concourse.bacc as bacc
import concourse.mybir as mybir
import concourse.tile as tile
from concourse import bass_utils
from concourse.bass import ds, ts

F32 = mybir.dt.float32
F32R = mybir.dt.float32r
EXP = mybir.ActivationFunctionType.Exp

B, S, D = 2, 2048, 1024
H, DK = 16, 64
P = 128
KS = D // P          # 8 contraction subtiles for d=1024
HL = 4               # heads per core
DL = HL * DK         # 256 local d-dims per core
NCI = S // 512       # 4 query chunks
NJT = S // P         # 16 key tiles
N_CORES = 8

_CACHE = {}


def _build(causal: bool):
    nc = bacc.Bacc("TRN2", target_bir_lowering=False, debug=False,
                   num_devices=N_CORES)

    xT_d = nc.dram_tensor("xT", [P, KS, S], F32R, kind="ExternalInput")
    wqk_d = nc.dram_tensor("wqk", [P, KS, 2 * DL], F32R, kind="ExternalInput")
    wv_d = nc.dram_tensor("wv", [P, KS, DL], F32R, kind="ExternalInput")
    wo_d = nc.dram_tensor("wo", [P, 2, D], F32R, kind="ExternalInput")
    if causal:
        # 2 patterns of [128, 2, 512]: diag key-tile pairs vs query chunk
        keep_d = nc.dram_tensor("keep", [P, 2, 2, 512], F32,
                                kind="ExternalInput")
    else:
        keep_d = nc.dram_tensor("keep", [P, NJT, S], F32,
                                kind="ExternalInput")
    out_d = nc.dram_tensor("partialT", [D, S], F32, kind="ExternalOutput")

    with tile.TileContext(nc) as tc:
        with tc.tile_pool(name="persist", bufs=1) as pp:
            qT = pp.tile([P, 2, S], F32R)
            kT = pp.tile([P, 2, S], F32R)
            v_aug = pp.tile([P, NJT, 65 * HL], F32R)
            wo_sb = pp.tile([P, 2, D], F32R)
            attn_outT = pp.tile([P, 2, S], F32R)
            bias_sb = pp.tile([P, 1], F32)
            nc.vector.memset(bias_sb[:], -6.0)
            ones_stage = pp.tile([P, DK], F32)
            nc.vector.memset(ones_stage[:], 1.0)
            for h in range(HL):
                nc.gpsimd.dma_start(v_aug[:, :, 65 * h + DK],
                                    ones_stage[:, 0:NJT])
            if causal:
                keep_sb = pp.tile([P, 2, 2, 512], F32)
                nc.sync.dma_start(keep_sb[:], keep_d[:])

            # ---- Inputs + V projection (k-outer, overlaps input DMA) ----
            ip = tc.alloc_tile_pool(name="inp", bufs=1)
            xT_sb = ip.tile([P, KS, S], F32R)
            wqk_sb = ip.tile([P, KS, 2 * DL], F32R)
            with tc.tile_pool(name="ipv", bufs=1) as ipv, \
                 tc.tile_pool(name="bp", bufs=8, space="PSUM") as bp:
                wv_sb = ipv.tile([P, KS, DL], F32R)
                for k in range(KS):
                    nc.sync.dma_start(wv_sb[:, k, :], wv_d[:, k, :])
                    nc.sync.dma_start(xT_sb[:, k, :], xT_d[:, k, :])
                for k in range(KS):
                    nc.sync.dma_start(wqk_sb[:, k, :], wqk_d[:, k, :])
                nc.sync.dma_start(wo_sb[:], wo_d[:])

                for half in range(2):
                    sts = range(8 * half, 8 * half + 8)
                    pssv = {st: bp.tile([P, DL], F32, tag="b",
                                        name=f"v_{st}")
                            for st in sts}
                    for k in range(KS):
                        for st in sts:
                            nc.tensor.matmul(
                                pssv[st][:],
                                xT_sb[:, k, ts(st, P)],
                                wv_sb[:, k, :],
                                start=(k == 0), stop=(k == KS - 1))
                    for st in sts:
                        for h in range(HL):
                            nc.scalar.copy(v_aug[:, st, ds(65 * h, DK)],
                                           pssv[st][:, ts(h, DK)])

            # ---- Attention + Wo; qk(ci+1) interleaved into attention(ci) ----
            with tc.tile_pool(name="qkp", bufs=2, space="PSUM") as qkp, \
                 tc.tile_pool(name="scp", bufs=2, space="PSUM") as scp, \
                 tc.tile_pool(name="avp", bufs=2, space="PSUM") as avp, \
                 tc.tile_pool(name="ptp", bufs=5 if causal else 4) as ptp, \
                 tc.tile_pool(name="keepp", bufs=4) as keepp, \
                 tc.tile_pool(name="osp", bufs=2) as osp, \
                 tc.tile_pool(name="smp", bufs=3) as smp:

                def emit_qk_group(sc, mc, on_act):
                    ps = qkp.tile([P, 512], F32, tag="q",
                                  name=f"qk_{sc}_{mc}")
                    for k in range(KS):
                        nc.tensor.matmul(
                            ps[:],
                            wqk_sb[:, k, ts(mc, P)],
                            xT_sb[:, k, ts(sc, 512)],
                            start=(k == 0), stop=(k == KS - 1))
                    # mc 0,1 -> q sub 0,1 ; mc 2,3 -> k sub 0,1
                    dst = qT if mc < 2 else kT
                    eng = nc.scalar if on_act else nc.vector
                    if on_act:
                        nc.scalar.copy(dst[:, mc % 2, ts(sc, 512)], ps[:])
                    else:
                        nc.vector.tensor_copy(dst[:, mc % 2, ts(sc, 512)],
                                              ps[:])

                def attention_gen(ci, keep_ci):
                    """Yields after each j2-pair so qk work can interleave."""
                    njt2 = 2 * ci + 2 if causal else NJT // 2
                    for hp in range(2):     # head pairs (2*hp, 2*hp+1)
                        av_pair = [avp.tile([65, 512], F32, tag="av",
                                            name=f"av_{ci}_{hp}_{i}")
                                   for i in range(2)]

                        def emit_av(hh, j2, pt):
                            h = 2 * hp + hh
                            for u in range(2):
                                nc.tensor.matmul(
                                    av_pair[hh][:],
                                    v_aug[:, 2 * j2 + u, ds(65 * h, 65)],
                                    pt[:, u, :],
                                    start=(j2 == 0 and u == 0),
                                    stop=(j2 == njt2 - 1 and u == 1))

                        pend = []
                        for j2 in range(njt2):  # key-tile pairs
                            for hh in range(2):  # head in pair: base 64*hh
                                base = 64 * hh
                                sp = scp.tile([P, 2, 512], F32, tag="sc")
                                for u in range(2):
                                    nc.tensor.matmul(
                                        sp[:, u, :],
                                        kT[ds(base, DK), hp,
                                           ts(2 * j2 + u, P)],
                                        qT[ds(base, DK), hp, ts(ci, 512)],
                                        start=True, stop=True)
                                pt = ptp.tile([P, 2, 512], F32R, tag="p")
                                nc.scalar.activation(pt[:], sp[:], EXP,
                                                     bias=bias_sb[:])
                                if causal:
                                    if j2 >= 2 * ci:
                                        nc.vector.tensor_mul(
                                            pt[:], pt[:],
                                            keep_sb[:, j2 - 2 * ci, :, :])
                                else:
                                    if hh == 0:
                                        keep_blk = keepp.tile(
                                            [P, 2, 512], F32, tag="kb")
                                        nc.sync.dma_start(
                                            keep_blk[:],
                                            keep_d[:, ds(2 * j2, 2),
                                                   ds(ci * 512, 512)])
                                    nc.vector.tensor_mul(pt[:], pt[:],
                                                         keep_blk[:])
                                pend.append((hh, j2, pt))
                            while len(pend) > 2:
                                emit_av(*pend.pop(0))
                            yield
                        for item in pend:
                            emit_av(*item)
                        for hh in range(2):
                            h = 2 * hp + hh
                            av = av_pair[hh]
                            sums = smp.tile([1, 512], F32, tag="sums")
                            nc.vector.tensor_scalar_add(sums[:],
                                                        av[DK:DK + 1, :],
                                                        1e-37)
                            rec = smp.tile([1, 512], F32, tag="rec")
                            nc.vector.reciprocal_approx_fast(rec[:], sums[:])
                            bc_sb = smp.tile([DK, 512], F32, tag="bcs")
                            nc.gpsimd.partition_broadcast(bc_sb[:], rec[:])
                            nc.vector.tensor_mul(
                                attn_outT[ds(64 * (h % 2), DK), h // 2,
                                          ts(ci, 512)],
                                av[0:DK, :], bc_sb[:])
                        yield

                def emit_wo_block(ci, ec):
                    wps = qkp.tile([P, 512], F32, tag="q",
                                   name=f"wo_{ci}_{ec}")
                    for k in range(2):
                        nc.tensor.matmul(
                            wps[:],
                            wo_sb[:, k, ts(ec, P)],
                            attn_outT[:, k, ts(ci, 512)],
                            start=(k == 0), stop=(k == 1))
                    out_sb = osp.tile([P, 512], F32, tag="osb")
                    nc.vector.tensor_copy(out_sb[:], wps[:])
                    nc.sync.dma_start(out_d[ts(ec, P), ts(ci, 512)],
                                      out_sb[:])

                def emit_wo(ci):
                    for ec in range(KS):
                        emit_wo_block(ci, ec)

                if causal:
                    # qk(0) upfront (copies on ACT: attention not running yet)
                    for mc in range(4):
                        emit_qk_group(0, mc, on_act=True)
                    for ci in range(NCI):
                        gen = attention_gen(ci, None)
                        # interleave qk(ci+1) + wo(ci-1) into attention(ci)
                        fill = []
                        if ci > 0:
                            fill += [("wo", ci - 1, ec) for ec in range(KS)]
                        if ci + 1 < NCI:
                            fill += [("qk", ci + 1, mc) for mc in range(4)]
                        fill = fill[1::2] + fill[0::2]  # spread types
                        for _ in gen:
                            if fill:
                                kind, a, b = fill.pop(0)
                                if kind == "qk":
                                    emit_qk_group(a, b, on_act=False)
                                else:
                                    emit_wo_block(a, b)
                        for kind, a, b in fill:
                            if kind == "qk":
                                emit_qk_group(a, b, on_act=False)
                            else:
                                emit_wo_block(a, b)
                    emit_wo(NCI - 1)
                else:
                    for sc in range(4):
                        for mc in range(4):
                            emit_qk_group(sc, mc, on_act=True)
                    for ci in range(NCI):
                        for _ in attention_gen(ci, None):
                            pass
                        emit_wo(ci)
            ip.release()

    nc.compile()
    return nc


def _get(causal: bool):
    if causal not in _CACHE:
        _CACHE[causal] = _build(causal)
    return _CACHE[causal]


def _tile_p(a2d):
    """[R, C] -> [128, R//128, C] with row r at (partition r%128, sub r//128)."""
    r, c = a2d.shape
    return np.ascontiguousarray(
        a2d.reshape(r // P, P, c).transpose(1, 0, 2))


def _causal_patterns():
    """keep[jj, t2, u, ii] for diagonal key-tile-pair t2 (pattern for
    j-tile 2*t2+u within the diag group): keep = ii >= 128*(2*t2+u) + jj."""
    jj = np.arange(P)[:, None, None, None]
    t2 = np.arange(2)[None, :, None, None]
    u = np.arange(2)[None, None, :, None]
    ii = np.arange(512)[None, None, None, :]
    return (ii >= P * (2 * t2 + u) + jj).astype(np.float32)


def _make_in_maps(x, mask, W_q, W_k, W_v, W_o, causal):
    x = np.asarray(x, dtype=np.float32)
    scale = 1.0 / np.sqrt(np.float32(DK))
    if causal:
        keep_host = np.ascontiguousarray(_causal_patterns())
    else:
        keepT = (~np.asarray(mask[0, 0])).astype(np.float32).T
        keep_host = _tile_p(np.ascontiguousarray(keepT))
    in_maps = []
    for c in range(N_CORES):
        b, g = c // 4, c % 4
        sl = slice(g * DL, (g + 1) * DL)
        xT = np.ascontiguousarray(x[b].T)
        # head h -> partitions 64*(h%2), sub h//2: row order within a
        # 256-row slice must be [h0, h1] sub 0 | [h2, h3] sub 1 -> natural.
        wqk = np.concatenate([np.asarray(W_q)[sl] * scale,
                              np.asarray(W_k)[sl]], axis=0).T
        in_maps.append({
            "xT": _tile_p(xT),
            "wqk": _tile_p(np.ascontiguousarray(wqk.astype(np.float32))),
            "wv": _tile_p(np.ascontiguousarray(
                np.asarray(W_v, dtype=np.float32)[sl].T)),
            "wo": _tile_p(np.ascontiguousarray(
                np.asarray(W_o, dtype=np.float32)[:, sl].T)),
            "keep": keep_host,
        })
    return in_maps


def run(x, mask, W_q, W_k, W_v, W_o, trace=False, trace_cores=None):
    mask2d = np.asarray(mask)[0, 0]
    causal = bool(np.array_equal(
        mask2d, ~np.tril(np.ones((S, S), dtype=bool))))
    nc = _get(causal)
    in_maps = _make_in_maps(x, mask, W_q, W_k, W_v, W_o, causal)
    kwargs = {}
    if trace:
        kwargs = dict(trace=True, trace_cores=trace_cores or [0])
    res = bass_utils.run_bass_kernel_spmd(
        nc, in_maps, core_ids=list(range(N_CORES)), **kwargs)
    outs = []
    for b in range(B):
        outT_b = res.results[4 * b]["partialT"].astype(np.float32).copy()
        for g in range(1, 4):
            outT_b += res.results[4 * b + g]["partialT"]
        outs.append(outT_b.T)
    return np.stack(outs).astype(np.float32), res


def kernel(x, mask, W_q, W_k, W_v, W_o):
    out, _ = run(x, mask, W_q, W_k, W_v, W_o, trace=False)
    return out
